# revision 1
# baseline (speedup 1.0000x reference)
"""Gumbel-Sinkhorn straight-through kernel for Trainium2 (raw Bass, manual sems).

Math: the reference computes, per sample matrix, L = (sigmoid(gamma)+noise)/temp,
then 20 iterations of row-logsumexp-subtract / col-logsumexp-subtract, and
returns exp(result).  In linear space that is exactly Sinkhorn scaling:
    X0 = exp(L - rowmax-ish shift)      (shift cancels in the first row norm)
    repeat 20x:  X /= rowsum(X);  X /= colsum(X)
which is what runs on device (fp32 throughout; the shift keeps exp in range).

Sharding: pure data parallel over samples -> 1024 per core, SPMD on 8 cores.

Per-core layout: two resident halves X_h[128, 256, 64] fp32 in SBUF.
Partition p = (hh, i): two blocks of 64 matrix-rows; free = (seg, j).
Sample s = h*512 + hh*256 + seg.

Engines, per half-iteration:
  DVE  rowscale X *= A (A bcast over j)     - 1x fp32 tensor_tensor
  PE   colsums: block-diag-ones lhsT @ X    - output replicated over i (PSUM)
  ACT  B = exp(-ln(colsum))                 - reciprocal (ACT Reciprocal is
                                              banned in bass; exp/ln share one
                                              activation table set)
  DVE  colscale X[:, chunk] *= B
  PE   rowsums: 64 accumulating identity-weighted matmuls (one per j)
  ACT  A' = exp(-ln(rowsum))
DVE (2 full passes/iter at 1 elem/cycle/lane) is the bottleneck; PE and ACT
hide under it.  Raw Bass (not Tile) because this toolchain's walrus supports
only a single sync-wait command per compute instruction - Tile's multi-wait
tail drain cannot compile, while manual sems give true transitivity through
semaphore chains with one wait per instruction.
"""

import sys

if "/opt/trn_rl_repo" not in sys.path:
    sys.path.insert(0, "/opt/trn_rl_repo")

import numpy as np

N = 64
ITERS = 20
TEMP = 0.1
NUM_SAMPLES = 8192
NCORES = 8
S_PER_CORE = NUM_SAMPLES // NCORES  # 1024

_PROGRAM_CACHE = {}


def _bc(ap, idx, count):
    """AP with a stride-0 (broadcast) free dim of `count` inserted at free
    position idx."""
    import concourse.bass as bass

    dims = list(ap.ap)
    dims.insert(1 + idx, [0, count])
    return bass.AP(tensor=ap.tensor, offset=ap.offset, ap=dims)


def build_program(s_per_core=S_PER_CORE, sub_segs=16, iters=ITERS, nb=4):
    from contextlib import ExitStack

    import concourse.bass as bass
    from concourse import mybir

    f32 = mybir.dt.float32
    AF = mybir.ActivationFunctionType

    assert s_per_core % 4 == 0
    half = s_per_core // 2
    nseg = half // 2
    assert nseg % sub_segs == 0
    nsub = nseg // sub_segs
    assert sub_segs % 8 == 0
    mm_per_sub = sub_segs // 8  # one N=512 fp32 matmul covers 8 segs

    nc = bass.Bass()
    noise_d = nc.dram_tensor("noise", [s_per_core, N, N], f32, kind="ExternalInput")
    consts_d = nc.dram_tensor("consts", [128, N + 256], f32, kind="ExternalInput")
    out_d = nc.dram_tensor("out", [s_per_core, N, N], f32, kind="ExternalOutput")

    def dram_ap(tensor_d, h, hh):
        base = (h * half + hh * nseg) * N * N
        return bass.AP(
            tensor=tensor_d.tensor if hasattr(tensor_d, "tensor") else tensor_d,
            offset=base,
            ap=[[N, N], [N * N, nseg], [1, N]],
        )

    # ---------------- tick schedules (prefix counts per engine) ----------
    # DVE: pre h: [redmax, ttsub, ttmulE]; iter (t,h): [rowscale, colscale*nsub]
    dve_ttsub = {h: 3 * h + 2 for h in range(2)}
    dve_ttmulE = {h: 3 * h + 3 for h in range(2)}

    def dve_rowscale(t, h):
        return 6 + (2 * t + h) * (1 + nsub) + 1

    def dve_colscale(t, h, n):
        return 6 + (2 * t + h) * (1 + nsub) + 2 + n

    # ACT: [exp0, exp1, lnR0, expA0, lnR1, expA1] then per (t,h):
    #      [(lnC, expB)*nsub, (t<last: lnR, expA)]
    act_exp = {0: 1, 1: 2}
    act_expA_pre = {0: 4, 1: 6}

    # simulate ACT counter
    act_expB = {}
    act_expA = {}
    _a = 6
    for _t in range(iters):
        for _h in range(2):
            for _n in range(nsub):
                _a += 1  # lnC
                _a += 1  # expB
                act_expB[(_t, _h, _n)] = _a
            if _t < iters - 1:
                _a += 1  # lnR
                _a += 1  # expA
                act_expA[(_t + 1, _h)] = _a
    act_expA.update({(0, 0): act_expA_pre[0], (0, 1): act_expA_pre[1]})

    # PE: [rowsums_pre0, rowsums_pre1]; per (t,h): [colsum*nsub, (t<last: rowsums)]
    pe_rowsum = {("pre", 0): 1, ("pre", 1): 2}
    pe_colsum = {}
    _p = 2
    for _t in range(iters):
        for _h in range(2):
            for _n in range(nsub):
                _p += 1
                pe_colsum[(_t, _h, _n)] = _p
            if _t < iters - 1:
                _p += 1
                pe_rowsum[(_t, _h)] = _p

    with ExitStack() as ctx:
        e = ctx.enter_context
        X = [e(nc.sbuf_tensor(f"x{h}", [128, nseg, N], f32)) for h in range(2)]
        A = [e(nc.sbuf_tensor(f"a{h}", [128, nseg], f32)) for h in range(2)]
        consts = e(nc.sbuf_tensor("consts_sb", [128, N + 256], f32))
        B = [
            e(nc.sbuf_tensor(f"b{k}", [128, sub_segs, N], f32)) for k in range(nb)
        ]
        C = [e(nc.psum_tensor(f"c{k}", [128, sub_segs, N], f32)) for k in range(2)]
        R = [e(nc.psum_tensor(f"r{h}", [128, nseg], f32)) for h in range(2)]

        sem_in_c = e(nc.semaphore("sem_in_c"))
        sem_in_h = [e(nc.semaphore(f"sem_in_h{h}")) for h in range(2)]
        sem_out = e(nc.semaphore("sem_out"))
        sem_dve = e(nc.semaphore("sem_dve"))
        sem_act = e(nc.semaphore("sem_act"))
        sem_pe = e(nc.semaphore("sem_pe"))

        e10sg = consts[:, 0:N]
        ident = consts[:, N : N + 128]
        bd = consts[:, N + 128 : N + 256]

        with nc.Block() as block:

            @block.sync
            def _(sync):
                sync.dma_start(out=consts[:, :], in_=consts_d[:, :]).then_inc(
                    sem_in_c, 16
                )
                for h in range(2):
                    for hh in range(2):
                        sync.dma_start(
                            out=X[h][hh * 64 : (hh + 1) * 64, :, :],
                            in_=dram_ap(noise_d, h, hh),
                        ).then_inc(sem_in_h[h], 16)
                for h in range(2):
                    sync.wait_ge(sem_dve, dve_colscale(iters - 1, h, nsub - 1))
                    for hh in range(2):
                        sync.dma_start(
                            out=dram_ap(out_d, h, hh),
                            in_=X[h][hh * 64 : (hh + 1) * 64, :, :],
                        ).then_inc(sem_out, 16)
                sync.wait_ge(sem_out, 64)

            @block.vector
            def _(vector):
                # dve self-tick: explicit same-engine ordering (the sim's race
                # detector does not assume the DVE per-op drain serializes)
                dc = [0]

                def dve_self_wait():
                    if dc[0]:
                        vector.wait_ge(sem_dve, dc[0])

                def dve_inc(inst):
                    inst.then_inc(sem_dve, 1)
                    dc[0] += 1

                for h in range(2):
                    vector.wait_ge(sem_in_h[h], 32)
                    # M = rowmax(noise) into A[h] (scratch use)
                    dve_self_wait()
                    dve_inc(nc.vector.reduce_max(
                        out=A[h][:, :], in_=X[h][:, :, :], axis=mybir.AxisListType.X
                    ))
                    # X -= M (bcast over j)
                    dve_self_wait()
                    dve_inc(nc.vector.tensor_sub(
                        X[h][:, :, :], X[h][:, :, :], _bc(A[h][:, :], 1, N)
                    ))
                    if h == 0:
                        vector.wait_ge(sem_in_c, 16)
                    # after ACT exp: X *= exp(10*sigmoid(gamma)) (bcast over seg)
                    vector.wait_ge(sem_act, act_exp[h])
                    dve_inc(nc.vector.tensor_mul(
                        X[h][:, :, :], X[h][:, :, :], _bc(e10sg, 0, nseg)
                    ))
                for t in range(iters):
                    for h in range(2):
                        vector.wait_ge(sem_act, act_expA[(t, h)])
                        dve_self_wait()
                        dve_inc(nc.vector.tensor_mul(
                            X[h][:, :, :], X[h][:, :, :], _bc(A[h][:, :], 1, N)
                        ))
                        for n in range(nsub):
                            vector.wait_ge(sem_act, act_expB[(t, h, n)])
                            dve_self_wait()
                            s0 = n * sub_segs
                            dve_inc(nc.vector.tensor_mul(
                                X[h][:, s0 : s0 + sub_segs, :],
                                X[h][:, s0 : s0 + sub_segs, :],
                                B[n % nb][:, :, :],
                            ))

            @block.scalar
            def _(scalar):
                ac = [0]

                def act_self_wait():
                    if ac[0]:
                        scalar.wait_ge(sem_act, ac[0])

                def act_inc(inst):
                    inst.then_inc(sem_act, 1)
                    ac[0] += 1

                for h in range(2):
                    scalar.wait_ge(sem_dve, dve_ttsub[h])
                    act_self_wait()
                    act_inc(nc.scalar.activation(
                        out=X[h][:, :, :], in_=X[h][:, :, :], func=AF.Exp, scale=10.0
                    ))
                for h in range(2):
                    scalar.wait_ge(sem_pe, pe_rowsum[("pre", h)])
                    act_self_wait()
                    act_inc(nc.scalar.activation(
                        out=R[h][:, :], in_=R[h][:, :], func=AF.Ln
                    ))
                    act_self_wait()
                    act_inc(nc.scalar.activation(
                        out=A[h][:, :], in_=R[h][:, :], func=AF.Exp, scale=-1.0
                    ))
                # track last DVE colscale tick that read each B buffer
                b_last_read = [0] * nb
                for t in range(iters):
                    for h in range(2):
                        for n in range(nsub):
                            scalar.wait_ge(sem_pe, pe_colsum[(t, h, n)])
                            if b_last_read[n % nb]:
                                scalar.wait_ge(sem_dve, b_last_read[n % nb])
                            act_self_wait()
                            act_inc(nc.scalar.activation(
                                out=C[n % 2][:, :, :],
                                in_=C[n % 2][:, :, :],
                                func=AF.Ln,
                            ))
                            act_self_wait()
                            act_inc(nc.scalar.activation(
                                out=B[n % nb][:, :, :],
                                in_=C[n % 2][:, :, :],
                                func=AF.Exp,
                                scale=-1.0,
                            ))
                            b_last_read[n % nb] = dve_colscale(t, h, n)
                        if t < iters - 1:
                            scalar.wait_ge(sem_pe, pe_rowsum[(t, h)])
                            act_self_wait()
                            act_inc(nc.scalar.activation(
                                out=R[h][:, :], in_=R[h][:, :], func=AF.Ln
                            ))
                            act_self_wait()
                            act_inc(nc.scalar.activation(
                                out=A[h][:, :], in_=R[h][:, :], func=AF.Exp, scale=-1.0
                            ))

            @block.tensor
            def _(tensor):
                tensor.wait_ge(sem_in_c, 16)

                def rowsums(h):
                    for j in range(N):
                        mm = nc.tensor.matmul(
                            R[h][:, :],
                            ident,
                            X[h][:, :, j],
                            start=(j == 0),
                            stop=(j == N - 1),
                        )
                    mm.then_inc(sem_pe, 1)

                for h in range(2):
                    tensor.wait_ge(sem_dve, dve_ttmulE[h])
                    rowsums(h)
                # last ACT expB tick that read each C buffer
                c_last_read = [0, 0]
                for t in range(iters):
                    for h in range(2):
                        tensor.wait_ge(sem_dve, dve_rowscale(t, h))
                        for n in range(nsub):
                            if c_last_read[n % 2]:
                                tensor.wait_ge(sem_act, c_last_read[n % 2])
                            s0 = n * sub_segs
                            for m in range(mm_per_sub):
                                mm = nc.tensor.matmul(
                                    C[n % 2][:, m * 8 : (m + 1) * 8, :],
                                    bd,
                                    X[h][:, s0 + m * 8 : s0 + (m + 1) * 8, :],
                                    start=True,
                                    stop=True,
                                )
                            mm.then_inc(sem_pe, 1)
                            c_last_read[n % 2] = act_expB[(t, h, n)]
                        if t < iters - 1:
                            tensor.wait_ge(sem_dve, dve_colscale(t, h, nsub - 1))
                            rowsums(h)

    return nc


def host_constants(gamma):
    """[128, 64+256] packed: exp(10*sigmoid(gamma)) | identity | block-diag."""
    sg = 1.0 / (1.0 + np.exp(-gamma.astype(np.float64)))
    e64 = np.exp(sg / TEMP).astype(np.float32)
    e10sg = np.concatenate([e64, e64], axis=0)
    ident = np.eye(128, dtype=np.float32)
    bdiag = np.kron(np.eye(2, dtype=np.float32), np.ones((64, 64), np.float32))
    return np.concatenate([e10sg, ident, bdiag], axis=1)


def kernel(gamma: np.ndarray, gumbel_noise: np.ndarray) -> np.ndarray:
    from concourse.bass_utils import run_bass_kernel_spmd

    gamma = np.asarray(gamma, dtype=np.float32)
    noise = np.asarray(gumbel_noise, dtype=np.float32)
    s = noise.shape[0]
    s_per_core = s // NCORES
    if s_per_core not in _PROGRAM_CACHE:
        _PROGRAM_CACHE[s_per_core] = build_program(s_per_core=s_per_core)
    nc = _PROGRAM_CACHE[s_per_core]

    consts = host_constants(gamma)
    in_maps = []
    for c in range(NCORES):
        shard = np.ascontiguousarray(noise[c * s_per_core : (c + 1) * s_per_core])
        in_maps.append({"noise": shard, "consts": consts})
    res = run_bass_kernel_spmd(nc, in_maps, list(range(NCORES)))
    out = np.concatenate([r["out"] for r in res.results], axis=0)
    return out.astype(np.float32)



# revision 46
# speedup vs baseline: 1.5008x; 1.5008x over previous
"""Gumbel-Sinkhorn kernel for Trainium2 (raw Bass, manual sems) — v2.

Math: per sample, L = (sigmoid(gamma) + noise)/temp, then 20 iterations of
row-logsumexp-subtract / col-logsumexp-subtract, output exp(result).  In
linear space that is Sinkhorn scaling of X0 = exp(L - S) (S = 80 constant
shift; safe: data exponent range [-24.4, 144.4], per-row max >= 20.9, so
sums stay below fp32 max and no significant entry underflows bf16).

v2 design (vs v1 baseline):
  * X stored bf16 with free layout (j-outer, seg-inner).  Both elementwise
    passes per iteration then qualify for the DVE 2x_1p mode (2-byte dtype,
    stride-1 innermost on every operand; the broadcast operand puts its
    stride-0 dim outermost).  fp16 is impossible: entries that end up O(1)
    dip to ~1e-17 mid-iteration, below fp16 range.
  * Col normalization is a direct DVE tensor_tensor DIVIDE by the PE colsum
    (converted fp32->bf16 by ACT Copy), instead of v1's exp(-ln(colsum))
    multiply.  That removes the 64x-replicated ln/exp stream that made ACT
    the second-slowest engine, leaving ACT with compact ln/exp (rowsums)
    plus PSUM->bf16 copies.
  * All loop matmuls are bf16 (1 cycle/row vs fp32's 4).
  * The exp is a single ACT pass per quarter with scale=10, bias=-80;
    exp(10*sigmoid(gamma)) is folded in as a separate elementwise multiply
    (optionally on GPSIMD to keep DVE free).
  * The 20th col normalization divides straight from PSUM (fp32, 1x) and
    writes fp16 to a (seg, j)-natural buffer so the output DMA is contiguous
    and half-width; the host casts fp16->fp32.

Iteration structure: pre-phase does exp, rowsums0, rowscale0, colsums0 and
col-divide0 per 256-sample quarter (pipelined with the input DMA); the main
loop runs t = 0..18 of [rowsums, invert, rowscale, colsums(chunked),
convert, divide], with t = 18 doing the final divide to the output buffer.

Sharding: pure data parallel over samples -> 1024 per core, SPMD on 8 cores.
"""

import sys

if "/opt/trn_rl_repo" not in sys.path:
    sys.path.insert(0, "/opt/trn_rl_repo")

import numpy as np

N = 64
ITERS = 20
TEMP = 0.1
SHIFT = 80.0
NUM_SAMPLES = 8192
NCORES = 8
S_PER_CORE = NUM_SAMPLES // NCORES  # 1024
NSEG = 256          # samples per (half, hh-block)
NQ = 4              # quarters per half
QSEG = NSEG // NQ   # 64 segs per quarter

# chunking of the col path in the main loop: 16 chunks of 4 j-columns
LCH = 16
LJW = N // LCH      # 4
# chunking in the pre-phase quarters: 4 chunks of 16 j-columns x 64 segs
PCH = 4
PJW = N // PCH      # 16

USE_POOL_P2 = True       # exp(10*sigmoid(gamma)) multiply on GPSIMD
# middle chunks go to GPSIMD: early chunks gate the PE rowsum-partial stream,
# the tail chunk gates the next invert, so both stay on the faster engines
# Hardware rules discovered the hard way: GPSIMD cannot touch PSUM, and
# TensorTensor divide is not a valid DVE ISA op.  The col normalization is
# therefore invert-then-multiply: each chunk's colsum reciprocal comes from
# either DVE InstReciprocal (PSUM -> bf16 CB directly) or ACT ln/exp
# (in-place PSUM ln, then exp(-lnC) -> CB), and the multiply runs at DVE 2x
# or on GPSIMD from SBUF.
ACT_INV_SET = {0, 1, 3, 5, 7, 9, 13, 14, 15}   # chunks inverted via ACT ln/exp
POOL_MUL_SET = {1, 2, 4, 5, 6, 8, 9, 10, 12, 13, 14}   # multiply chunks on GPSIMD
RSP_LAG = 9                       # chunks the rowsum partials trail the divides

_PROGRAM_CACHE = {}
DEBUG_LABELS = {}


def _mk(base, extra_off, free_dims):
    """AP with base's partition dim, custom free dims ([stride, count] elems)."""
    import concourse.bass as bass

    return bass.AP(
        tensor=base.tensor,
        offset=base.offset + extra_off,
        ap=[list(base.ap[0])] + [list(d) for d in free_dims],
    )


def build_program(s_per_core=S_PER_CORE):
    from contextlib import ExitStack

    import concourse.bass as bass
    from concourse import mybir

    f32 = mybir.dt.float32
    bf16 = mybir.dt.bfloat16
    f16 = mybir.dt.float16
    AF = mybir.ActivationFunctionType
    DIV = mybir.AluOpType.divide
    MUL = mybir.AluOpType.mult

    assert s_per_core == 1024, "layout hardcoded for 1024 samples/core"

    nc = bass.Bass()
    # register a [128,1] const AP for the exp bias (only 0.0/1.0 are built in)
    _bias_t = nc.alloc_sbuf_tensor(f"const-f32-bias", [128, 1], f32)
    nc.gpsimd.memset(_bias_t.ap(), -SHIFT)
    nc.const_aps.aps[(f32, -SHIFT)] = _bias_t.ap()
    nc.all_engine_barrier()

    noise_d = nc.dram_tensor("noise", [s_per_core, N, N], f32, kind="ExternalInput")
    k16_d = nc.dram_tensor("consts16", [128, N + 256], bf16, kind="ExternalInput")
    # raw dump in the on-chip (h, p, j, seg) layout; host reorders.  32 KiB
    # contiguous runs keep the output DMA at full bandwidth (runs < 512 B
    # pay a 2x latency penalty in the DMA engines).
    out_d = nc.dram_tensor("out", [2, 128, N, NSEG], f16, kind="ExternalOutput")

    def noise_ap(h, hh, q):
        base = (h * 512 + hh * 256 + q * QSEG) * N * N
        return bass.AP(
            tensor=noise_d.tensor if hasattr(noise_d, "tensor") else noise_d,
            offset=base,
            ap=[[N, N], [N * N, QSEG], [1, N]],
        )

    def out_ap(h):
        base = h * 128 * N * NSEG
        return bass.AP(
            tensor=out_d.tensor if hasattr(out_d, "tensor") else out_d,
            offset=base,
            ap=[[N * NSEG, 128], [1, N * NSEG]],
        )

    # ------------------------------------------------------------------
    # planning: per-engine op lists with cross-engine tick waits
    # ------------------------------------------------------------------
    ENGINES = ("sync", "vector", "scalar", "tensor", "pool")

    class Plan:
        def __init__(self):
            self.ops = {e: [] for e in ENGINES}
            self.n = {e: 0 for e in ENGINES}

        def add(self, eng, emit, waits=(), label="", counted=True):
            self.ops[eng].append((emit, [w for w in waits if w is not None],
                                  label, counted))
            if counted:
                self.n[eng] += 1
            return self.n[eng]

    with ExitStack() as ctx:
        e = ctx.enter_context
        NCB = 8
        NCP = 3
        X = [e(nc.sbuf_tensor(f"x{h}", [128, N, NSEG], bf16)) for h in range(2)]
        OUTB = [e(nc.sbuf_tensor(f"ob{h}", [128, N, NSEG], f16)) for h in range(2)]
        NQB = [e(nc.sbuf_tensor(f"nq{b}", [128, QSEG, N], f32)) for b in range(2)]
        CB = [e(nc.sbuf_tensor(f"cb{b}", [128, LJW * NSEG], bf16))
              for b in range(NCB)]
        A = [e(nc.sbuf_tensor(f"a{h}", [128, NSEG], bf16)) for h in range(2)]
        K16 = e(nc.sbuf_tensor("k16", [128, N + 256], bf16))
        R = [e(nc.psum_tensor(f"r{h}", [128, NSEG], f32)) for h in range(2)]
        CP = [e(nc.psum_tensor(f"cp{b}", [128, LJW * NSEG], f32))
              for b in range(NCP)]

        e10sg = K16[:, 0:N]                # [128, 64]
        identb = K16[:, N : N + 128]       # [128, 128]
        bdb = K16[:, N + 128 : N + 256]    # [128, 128]

        sems = {
            "ink": e(nc.semaphore("sem_in_k")),
            "nq0": e(nc.semaphore("sem_nq0")),
            "nq1": e(nc.semaphore("sem_nq1")),
            "nv0": e(nc.semaphore("sem_nv0")),
            "nv1": e(nc.semaphore("sem_nv1")),
            "out": e(nc.semaphore("sem_out")),
            "outv": e(nc.semaphore("sem_outv")),
            "vector": e(nc.semaphore("sem_dve")),
            "scalar": e(nc.semaphore("sem_act")),
            "tensor": e(nc.semaphore("sem_pe")),
            "pool": e(nc.semaphore("sem_pool")),
        }

        P = Plan()

        # tick tables
        tE = {}      # ACT exp per quarter
        tP2 = {}     # gamma multiply per quarter -> (engine, tick)
        tI0 = {}     # pre invert exp per quarter
        tRSC0 = {}   # pre rowscale per quarter (DVE)
        tDV0 = {}    # pre divide per (Q, cc) (DVE)
        tINV = {}    # loop invert exp per (t, h) (ACT)
        tRSC = {}    # loop rowscale per (t, h) (DVE)
        tCV = {}     # loop convert per (t, h, c) (ACT)
        tDV = {}     # loop divide per (t, h, c) -> (engine, tick)
        cp_reader = [None] * NCP   # (engine, tick) that last read CP[b]
        cb_reader = [None] * NCB   # (engine, tick) that last read CB[b]

        XB = [X[h][:, :, :] for h in range(2)]     # base APs
        OB = [OUTB[h][:, :, :] for h in range(2)]

        # ---------------- global schedule walk -------------------------
        # sync: consts first
        P.add("sync", lambda: nc.sync.dma_start(out=K16[:, :], in_=k16_d[:, :])
              .then_inc(sems["ink"], 16))

        def plan_dma_in(Q):
            h, q = Q // 4, Q % 4
            w = [("scalar", tE[Q - 2])] if Q >= 2 else []
            for hh in range(2):
                def dma_in(h=h, hh=hh, q=q, b=Q % 2):
                    return nc.sync.dma_start(
                        out=NQB[b][hh * 64 : (hh + 1) * 64, :, :],
                        in_=noise_ap(h, hh, q),
                    ).then_inc(sems["nq%d" % (Q % 2)], 16)
                P.add("sync", dma_in, w)
                w = []

        # build ACT/DVE/PE/POOL streams quarter by quarter; ACT exp runs two
        # quarters ahead (ping-pong NQB).
        def plan_exp(Q):
            h, q = Q // 4, Q % 4
            # in: NQB[Q%2] [128, (seg 64), (j 64)]; out: X[h] quarter strided
            def emit(h=h, q=q, b=Q % 2):
                return nc.scalar.activation(
                    out=_mk(XB[h], q * QSEG, [[1, QSEG], [NSEG, N]]),
                    in_=NQB[b][:, :, :],
                    func=AF.Exp,
                    scale=1.0 / TEMP,
                    bias=-SHIFT,
                )
            tE[Q] = P.add("scalar", emit,
                          [("nq%d" % (Q % 2), 32 * (Q // 2 + 1))])

        def plan_quarter(Q):
            h, q = Q // 4, Q % 4
            qo = q * QSEG

            # P2: X_q *= e10sg  (j-major iteration; e10sg bcast over seg);
            # alternate GPSIMD/DVE so neither engine paces the pre-phase
            p2_pool = USE_POOL_P2 and (Q % 2 == 0)
            p2_eng = "pool" if p2_pool else "vector"
            def emit_p2(h=h, qo=qo, p2_pool=p2_pool):
                eng = nc.gpsimd if p2_pool else nc.vector
                xq = _mk(XB[h], qo, [[NSEG, N], [1, QSEG]])
                return eng.tensor_tensor(
                    out=xq, in0=xq,
                    in1=_mk(K16[:, 0:N], 0, [[1, N], [0, QSEG]]),
                    op=MUL,
                )
            tP2[Q] = (p2_eng, P.add(p2_eng, emit_p2,
                                    [("scalar", tE[Q]), ("ink", 16)]))

            # RS0: 64 accumulating matmuls -> R[h][:, qo:qo+QSEG]
            first = [("ink", 16), (tP2[Q][0], tP2[Q][1])]
            for j in range(N):
                def emit_rs0(h=h, qo=qo, j=j):
                    return nc.tensor.matmul(
                        R[h][:, qo : qo + QSEG],
                        identb,
                        _mk(XB[h], j * NSEG + qo, [[1, QSEG]]),
                        start=(j == 0),
                        stop=(j == N - 1),
                    )
                t_rs0 = P.add("tensor", emit_rs0, first)
                first = []

            # inv0: A_q = 1/R_q via DVE reciprocal (R0 up to ~2e31 exceeds the
            # ACT Ln domain of 2^64^0.5-ish; loop rowsums are tame)
            def emit_inv0(h=h, qo=qo):
                with nc.allow_low_precision("A0 feeds a bf16 multiply anyway"):
                    return nc.vector.reciprocal(
                        out=A[h][:, qo : qo + QSEG],
                        in_=R[h][:, qo : qo + QSEG],
                    )
            tI0[Q] = P.add("vector", emit_inv0, [("tensor", t_rs0)])

            # rsc0: X_q *= A_q (2x: j-major, seg-inner; A bcast over j)
            def emit_rsc0(h=h, qo=qo):
                xq = _mk(XB[h], qo, [[NSEG, N], [1, QSEG]])
                return nc.vector.tensor_tensor(
                    out=xq, in0=xq,
                    in1=_mk(A[h][:, :], qo, [[0, N], [1, QSEG]]),
                    op=MUL,
                )
            tRSC0[Q] = P.add("vector", emit_rsc0)

            # CS0 / cvt0 / div0 chunks: PJW=16 j-cols x QSEG=64 segs
            for cc in range(PCH):
                pb = cc % NCP
                cbb = cc % NCB
                w = [("vector", tRSC0[Q])] if cc == 0 else []
                if cp_reader[pb] is not None:
                    w.append(cp_reader[pb])
                for m in range(2):
                    def emit_cs0(h=h, qo=qo, cc=cc, m=m, pb=pb):
                        return nc.tensor.matmul(
                            _mk(CP[pb][:, :], m * 512, [[QSEG, 8], [1, QSEG]]),
                            bdb,
                            _mk(XB[h], (cc * PJW + 8 * m) * NSEG + qo,
                                [[NSEG, 8], [1, QSEG]]),
                            start=True, stop=True,
                        )
                    t_cs0 = P.add("tensor", emit_cs0, w)
                    w = []
                wcv = [("tensor", t_cs0)]
                if cb_reader[cbb] is not None:
                    wcv.append(cb_reader[cbb])
                if cc % 4 == 3:
                    def emit_ln0(pb=pb):
                        return nc.scalar.activation(
                            out=CP[pb][:, :], in_=CP[pb][:, :], func=AF.Ln,
                        )
                    P.add("scalar", emit_ln0, [("tensor", t_cs0)])
                    def emit_cv0(pb=pb, cbb=cbb):
                        return nc.scalar.activation(
                            out=CB[cbb][:, :], in_=CP[pb][:, :],
                            func=AF.Exp, scale=-1.0,
                        )
                    t_cv0 = P.add("scalar", emit_cv0, wcv)
                    inv_eng0 = "scalar"
                else:
                    def emit_cv0(pb=pb, cbb=cbb):
                        with nc.allow_low_precision("colsum recip to bf16"):
                            return nc.vector.reciprocal(
                                out=CB[cbb][:, :], in_=CP[pb][:, :],
                            )
                    t_cv0 = P.add("vector", emit_cv0, wcv)
                    inv_eng0 = "vector"
                cp_reader[pb] = (inv_eng0, t_cv0)
                def emit_dv0(h=h, qo=qo, cc=cc, cbb=cbb):
                    xq = _mk(XB[h], cc * PJW * NSEG + qo, [[NSEG, PJW], [1, QSEG]])
                    return nc.vector.tensor_tensor(
                        out=xq, in0=xq,
                        in1=_mk(CB[cbb][:, :], 0, [[QSEG, PJW], [1, QSEG]]),
                        op=MUL,
                    )
                tDV0[(Q, cc)] = P.add("vector", emit_dv0, [(inv_eng0, t_cv0)])
                cb_reader[cbb] = ("vector", tDV0[(Q, cc)])

        plan_dma_in(0)
        plan_exp(0)
        plan_dma_in(1)
        plan_exp(1)
        for Q in range(8):
            if Q + 2 < 8:
                plan_dma_in(Q + 2)
                plan_exp(Q + 2)
            plan_quarter(Q)

        # ---------------- main loop t = 0..18 --------------------------
        # Software-pipelined: rowsums for iteration t+1 are accumulated as
        # 4-matmul partials interleaved right behind iteration t's divides,
        # so the turnaround (last divide -> invert -> first rowscale) is
        # short and the two halves phase-shift against each other.
        act_inv_set = ACT_INV_SET
        pool_mul_set = POOL_MUL_SET
        tRSP = {}   # (t, h) -> PE tick of the last partial-rowsum matmul

        def plan_rsp(t, h, c, wdep):
            """Partial rowsums for iteration t: 4 matmuls for j in chunk c."""
            w = list(wdep)
            for jj in range(LJW):
                j = c * LJW + jj
                def emit_rsp(h=h, j=j, c=c, jj=jj):
                    return nc.tensor.matmul(
                        R[h][:, :], identb,
                        _mk(XB[h], j * NSEG, [[1, NSEG]]),
                        start=(c == 0 and jj == 0),
                        stop=(c == LCH - 1 and jj == LJW - 1),
                    )
                tick = P.add("tensor", emit_rsp, w, label=f"rsp{t},{h},{c}")
                w = []
            if c == LCH - 1:
                tRSP[(t, h)] = tick

        # rowsums for t=0 accumulate over the pre-phase divides
        for h in range(2):
            for c in range(LCH):
                plan_rsp(0, h, c, [("vector", tDV0[(h * 4 + 3, c // 4)])])

        # rowscale chunk widths in j-columns: small first chunk so the first
        # colsum (and with it the ACT convert stream) starts early
        RSCW = [4, 12, 16, 16, 16]
        RSCO = [0, 4, 16, 32, 48]          # cumulative j offsets
        def rsck_of(c):
            j = c * LJW
            for k in range(len(RSCW)):
                if j < RSCO[k] + RSCW[k]:
                    return k
            raise AssertionError

        for t in range(ITERS - 1):
            last = t == ITERS - 2
            for h in range(2):
                # invert: ln R in place, exp(-lnR) -> A (tame range in-loop)
                def emit_ln(h=h):
                    return nc.scalar.activation(
                        out=R[h][:, :], in_=R[h][:, :], func=AF.Ln,
                    )
                P.add("scalar", emit_ln, [("tensor", tRSP[(t, h)])], label=f"ln{t},{h}")
                def emit_inv(h=h):
                    return nc.scalar.activation(
                        out=A[h][:, :], in_=R[h][:, :], func=AF.Exp, scale=-1.0,
                    )
                tINV[(t, h)] = P.add("scalar", emit_inv, label=f"inv{t},{h}")

                w = [("scalar", tINV[(t, h)])]
                for k in range(len(RSCW)):
                    def emit_rsc(h=h, k=k):
                        xf = _mk(XB[h], RSCO[k] * NSEG, [[NSEG, RSCW[k]], [1, NSEG]])
                        return nc.vector.tensor_tensor(
                            out=xf, in0=xf,
                            in1=_mk(A[h][:, :], 0, [[0, RSCW[k]], [1, NSEG]]),
                            op=MUL,
                        )
                    tRSC[(t, h, k)] = P.add("vector", emit_rsc, w, label=f"rsc{t},{h},{k}")
                    w = []

                last_rsck = -1
                for c in range(LCH):
                    pb = c % NCP
                    cbb = c % NCB
                    w = []
                    if rsck_of(c) > last_rsck:
                        last_rsck = rsck_of(c)
                        w.append(("vector", tRSC[(t, h, last_rsck)]))
                    if cp_reader[pb] is not None:
                        w.append(cp_reader[pb])
                    for m in range(2):
                        def emit_cs(h=h, c=c, m=m, pb=pb):
                            return nc.tensor.matmul(
                                _mk(CP[pb][:, :], m * 512, [[NSEG, 2], [1, NSEG]]),
                                bdb,
                                _mk(XB[h], (c * LJW + 2 * m) * NSEG,
                                    [[NSEG, 2], [1, NSEG]]),
                                start=True, stop=True,
                            )
                        t_cs = P.add("tensor", emit_cs, w, label=f"cs{t},{h},{c}.{m}")
                        w = []

                    # destination: in-place X for t<18, OUT16 for t=18
                    if last:
                        dst = _mk(OB[h], c * LJW * NSEG, [[NSEG, LJW], [1, NSEG]])
                    else:
                        dst = _mk(XB[h], c * LJW * NSEG, [[NSEG, LJW], [1, NSEG]])
                    xin = _mk(XB[h], c * LJW * NSEG, [[NSEG, LJW], [1, NSEG]])
                    lbl = ("fdv" if last else "div") + f"{t},{h},{c}"

                    # invert the chunk's colsums into CB[cbb] (bf16)
                    wcv = [("tensor", t_cs)]
                    if cb_reader[cbb] is not None:
                        wcv.append(cb_reader[cbb])
                    if c in act_inv_set:
                        def emit_lnc(pb=pb):
                            return nc.scalar.activation(
                                out=CP[pb][:, :], in_=CP[pb][:, :], func=AF.Ln,
                            )
                        P.add("scalar", emit_lnc, [("tensor", t_cs)],
                              label=f"lnc{t},{h},{c}")
                        def emit_cv(pb=pb, cbb=cbb):
                            return nc.scalar.activation(
                                out=CB[cbb][:, :], in_=CP[pb][:, :],
                                func=AF.Exp, scale=-1.0,
                            )
                        t_cv = P.add("scalar", emit_cv, wcv, label=f"cvt{t},{h},{c}")
                        inv_eng = "scalar"
                    else:
                        def emit_cv(pb=pb, cbb=cbb):
                            with nc.allow_low_precision("colsum recip to bf16"):
                                return nc.vector.reciprocal(
                                    out=CB[cbb][:, :], in_=CP[pb][:, :],
                                )
                        t_cv = P.add("vector", emit_cv, wcv, label=f"cvt{t},{h},{c}")
                        inv_eng = "vector"
                    tCV[(t, h, c)] = (inv_eng, t_cv)
                    cp_reader[pb] = (inv_eng, t_cv)

                    mul_eng = "pool" if c in pool_mul_set else "vector"
                    def emit_dv(dst=dst, xin=xin, cbb=cbb, mul_eng=mul_eng):
                        eng = nc.gpsimd if mul_eng == "pool" else nc.vector
                        return eng.tensor_tensor(
                            out=dst, in0=xin,
                            in1=_mk(CB[cbb][:, :], 0, [[NSEG, LJW], [1, NSEG]]),
                            op=MUL,
                        )
                    tdv = P.add(mul_eng, emit_dv, [tCV[(t, h, c)]], label=lbl)
                    tDV[(t, h, c)] = (mul_eng, tdv)
                    cb_reader[cbb] = (mul_eng, tdv)

                    # interleave next iteration's partial rowsums far enough
                    # behind the divides that the CS->cvt->div pipeline depth
                    # (~6 chunks) never stalls the PE stream
                    if not last and c >= RSP_LAG:
                        plan_rsp(t + 1, h, c - RSP_LAG, [tDV[(t, h, c - RSP_LAG)]])
                if not last:
                    for cc2 in range(LCH - RSP_LAG, LCH):
                        plan_rsp(t + 1, h, cc2, [tDV[(t, h, cc2)]])

        # output DMA: one raw transfer per half
        for h in range(2):
            # wait the last divide on each engine (DVE odd, pool even chunks)
            w = [tDV[(ITERS - 2, h, LCH - 1)], tDV[(ITERS - 2, h, LCH - 2)]]
            def dma_out(h=h):
                return nc.sync.dma_start(
                    out=out_ap(h),
                    in_=OUTB[h][:, :, :],
                ).then_inc(sems["out"], 16)
            P.add("sync", dma_out, w)
        P.add("sync", lambda: None, [("out", 32)])

        # ------------------------------------------------------------------
        # emission
        # ------------------------------------------------------------------
        def emit_stream(handle, eng):
            own = sems.get(eng)
            waited = {}
            tick = 0
            self_order = eng in ("vector", "scalar", "pool")
            for emit, waits, label, counted in P.ops[eng]:
                for sname, val in waits:
                    if waited.get(sname, 0) >= val:
                        continue
                    handle.wait_ge(sems[sname], val)
                    waited[sname] = val
                if self_order and tick > 0 and waited.get(eng, 0) < tick:
                    handle.wait_ge(sems[eng], tick)
                    waited[eng] = tick
                inst = emit()
                if not counted:
                    continue
                tick += 1
                if eng != "sync" and inst is not None:
                    inst.then_inc(sems[eng], 1)
                    if label:
                        DEBUG_LABELS[inst.ins.name] = label

        with nc.Block() as block:

            @block.sync
            def _(sync):
                emit_stream(sync, "sync")

            @block.vector
            def _(vector):
                emit_stream(vector, "vector")

            @block.scalar
            def _(scalar):
                emit_stream(scalar, "scalar")

            @block.tensor
            def _(tensor):
                emit_stream(tensor, "tensor")

            if P.ops["pool"]:

                @block.gpsimd
                def _(pool):
                    emit_stream(pool, "pool")

    return nc


def host_constants():
    """[128, 64+256] bf16: e10sg placeholder | identity | block-diag."""
    import ml_dtypes

    ident = np.eye(128, dtype=np.float32)
    bdiag = np.kron(np.eye(2, dtype=np.float32), np.ones((64, 64), np.float32))
    return ident, bdiag


def pack_consts16(gamma):
    import ml_dtypes

    sg = 1.0 / (1.0 + np.exp(-gamma.astype(np.float64)))
    e10sg = np.exp(sg / TEMP).astype(np.float32)           # [64, 64]
    e10sg2 = np.concatenate([e10sg, e10sg], axis=0)        # [128, 64]
    ident, bdiag = host_constants()
    k = np.concatenate([e10sg2, ident, bdiag], axis=1)     # [128, 320]
    return k.astype(ml_dtypes.bfloat16)


def kernel(gamma: np.ndarray, gumbel_noise: np.ndarray) -> np.ndarray:
    from concourse.bass_utils import run_bass_kernel_spmd

    gamma = np.asarray(gamma, dtype=np.float32)
    noise = np.asarray(gumbel_noise, dtype=np.float32)
    s = noise.shape[0]
    s_per_core = s // NCORES
    if s_per_core not in _PROGRAM_CACHE:
        _PROGRAM_CACHE[s_per_core] = build_program(s_per_core=s_per_core)
    nc = _PROGRAM_CACHE[s_per_core]

    k16 = pack_consts16(gamma)
    in_maps = []
    for c in range(NCORES):
        shard = np.ascontiguousarray(noise[c * s_per_core : (c + 1) * s_per_core])
        in_maps.append({"noise": shard, "consts16": k16})
    res = run_bass_kernel_spmd(nc, in_maps, list(range(NCORES)))
    outs = []
    for r in res.results:
        buf = np.asarray(r["out"])  # [2, 128, 64, 256] = (h, (hh,i), j, seg)
        buf = buf.reshape(2, 2, 64, N, NSEG).transpose(0, 1, 4, 2, 3)
        outs.append(buf.reshape(s_per_core, N, N))
    return np.concatenate(outs, axis=0).astype(np.float32)


# revision 47
# speedup vs baseline: 1.5171x; 1.0109x over previous
"""Gumbel-Sinkhorn kernel for Trainium2 (raw Bass, manual sems) — v2.

Math: per sample, L = (sigmoid(gamma) + noise)/temp, then 20 iterations of
row-logsumexp-subtract / col-logsumexp-subtract, output exp(result).  In
linear space that is Sinkhorn scaling of X0 = exp(L - S) (S = 80 constant
shift; safe: data exponent range [-24.4, 144.4], per-row max >= 20.9, so
sums stay below fp32 max and no significant entry underflows bf16).

v2 design (vs v1 baseline, TimelineSim ~1.4 ms vs ~4.3 ms):
  * X stored bf16 with free layout (j-outer, seg-inner).  Both elementwise
    passes per iteration then qualify for the DVE 2x_1p mode (2-byte dtype,
    stride-1 innermost on every operand; the broadcast operand puts its
    stride-0 dim outermost).  fp16 is impossible: entries that end up O(1)
    dip to ~1e-17 mid-iteration, below fp16 range.
  * All loop matmuls are bf16 (1 cycle/row vs fp32's 4 on the PE).
  * Col normalization is invert-then-multiply, chunked 4 j-columns at a
    time and load-balanced across engines: each chunk's colsum reciprocal
    comes from DVE InstReciprocal (PSUM -> bf16, chunks not in ACT_INV_SET)
    or ACT ln + exp(-x) (ACT_INV_SET), and the multiply runs at DVE 2x or
    on GPSIMD (POOL_MUL_SET).  (TensorTensor divide is not a valid DVE ISA
    op, and GPSIMD cannot access PSUM — see memory notes.)
  * Rowsums for iteration t+1 are accumulated as 4-matmul partials
    interleaved RSP_LAG chunks behind iteration t's multiplies, so the
    turnaround (last multiply -> invert -> first rowscale chunk) is short;
    the two halves phase-shift against each other.
  * The exp is a single ACT pass per quarter with scale=10, bias=-80;
    exp(10*sigmoid(gamma)) is folded in as a separate multiply on
    GPSIMD/DVE (alternating per quarter).
  * Output is written fp16 in the on-chip layout and DMA'd as one raw
    32 KiB-run transfer per half (runs < 512 B pay a 2x DMA latency
    penalty); the host reorders and casts.

Iteration structure: pre-phase does exp, rowsums0, rowscale0, colsums0 and
col-norm0 per 256-sample quarter (pipelined with the input DMA); the main
loop runs t = 0..18 of [partial rowsums, invert, rowscale(chunked),
colsums(chunked), colsum-invert, multiply], with t = 18 writing the
multiplies to the output buffer.

Sharding: pure data parallel over samples -> 1024 per core, SPMD on 8 cores.
"""

import sys

if "/opt/trn_rl_repo" not in sys.path:
    sys.path.insert(0, "/opt/trn_rl_repo")

import numpy as np

N = 64
ITERS = 20
TEMP = 0.1
SHIFT = 80.0
NUM_SAMPLES = 8192
NCORES = 8
S_PER_CORE = NUM_SAMPLES // NCORES  # 1024
NSEG = 256          # samples per (half, hh-block)
NQ = 4              # quarters per half
QSEG = NSEG // NQ   # 64 segs per quarter

# chunking of the col path in the main loop: 16 chunks of 4 j-columns
LCH = 16
LJW = N // LCH      # 4
# chunking in the pre-phase quarters: 4 chunks of 16 j-columns x 64 segs
PCH = 4
PJW = N // PCH      # 16

USE_POOL_P2 = True       # exp(10*sigmoid(gamma)) multiply on GPSIMD
# middle chunks go to GPSIMD: early chunks gate the PE rowsum-partial stream,
# the tail chunk gates the next invert, so both stay on the faster engines
# Hardware rules discovered the hard way: GPSIMD cannot touch PSUM, and
# TensorTensor divide is not a valid DVE ISA op.  The col normalization is
# therefore invert-then-multiply: each chunk's colsum reciprocal comes from
# either DVE InstReciprocal (PSUM -> bf16 CB directly) or ACT ln/exp
# (in-place PSUM ln, then exp(-lnC) -> CB), and the multiply runs at DVE 2x
# or on GPSIMD from SBUF.
ACT_INV_SET = {0, 1, 3, 5, 7, 9, 13, 14, 15}   # chunks inverted via ACT ln/exp
POOL_MUL_SET = {1, 2, 4, 5, 6, 8, 9, 10, 12, 13, 14}   # multiply chunks on GPSIMD
RSP_LAG = 9                       # chunks the rowsum partials trail the divides

_PROGRAM_CACHE = {}
DEBUG_LABELS = {}


def _mk(base, extra_off, free_dims):
    """AP with base's partition dim, custom free dims ([stride, count] elems)."""
    import concourse.bass as bass

    return bass.AP(
        tensor=base.tensor,
        offset=base.offset + extra_off,
        ap=[list(base.ap[0])] + [list(d) for d in free_dims],
    )


def build_program(s_per_core=S_PER_CORE):
    from contextlib import ExitStack

    import concourse.bass as bass
    from concourse import mybir

    f32 = mybir.dt.float32
    bf16 = mybir.dt.bfloat16
    f16 = mybir.dt.float16
    AF = mybir.ActivationFunctionType
    DIV = mybir.AluOpType.divide
    MUL = mybir.AluOpType.mult

    assert s_per_core == 1024, "layout hardcoded for 1024 samples/core"

    nc = bass.Bass()
    # register a [128,1] const AP for the exp bias (only 0.0/1.0 are built in)
    _bias_t = nc.alloc_sbuf_tensor(f"const-f32-bias", [128, 1], f32)
    nc.gpsimd.memset(_bias_t.ap(), -SHIFT)
    nc.const_aps.aps[(f32, -SHIFT)] = _bias_t.ap()
    nc.all_engine_barrier()

    noise_d = nc.dram_tensor("noise", [s_per_core, N, N], f32, kind="ExternalInput")
    k16_d = nc.dram_tensor("consts16", [128, N + 256], bf16, kind="ExternalInput")
    # raw dump in the on-chip (h, p, j, seg) layout; host reorders.  32 KiB
    # contiguous runs keep the output DMA at full bandwidth (runs < 512 B
    # pay a 2x latency penalty in the DMA engines).
    out_d = nc.dram_tensor("out", [2, 128, N, NSEG], f16, kind="ExternalOutput")

    def noise_ap(h, hh, q):
        base = (h * 512 + hh * 256 + q * QSEG) * N * N
        return bass.AP(
            tensor=noise_d.tensor if hasattr(noise_d, "tensor") else noise_d,
            offset=base,
            ap=[[N, N], [N * N, QSEG], [1, N]],
        )

    def out_ap(h):
        base = h * 128 * N * NSEG
        return bass.AP(
            tensor=out_d.tensor if hasattr(out_d, "tensor") else out_d,
            offset=base,
            ap=[[N * NSEG, 128], [1, N * NSEG]],
        )

    # ------------------------------------------------------------------
    # planning: per-engine op lists with cross-engine tick waits
    # ------------------------------------------------------------------
    ENGINES = ("sync", "vector", "scalar", "tensor", "pool")

    class Plan:
        def __init__(self):
            self.ops = {e: [] for e in ENGINES}
            self.n = {e: 0 for e in ENGINES}

        def add(self, eng, emit, waits=(), label="", counted=True):
            self.ops[eng].append((emit, [w for w in waits if w is not None],
                                  label, counted))
            if counted:
                self.n[eng] += 1
            return self.n[eng]

    with ExitStack() as ctx:
        e = ctx.enter_context
        NCB = 8
        NCP = 3
        X = [e(nc.sbuf_tensor(f"x{h}", [128, N, NSEG], bf16)) for h in range(2)]
        OUTB = [e(nc.sbuf_tensor(f"ob{h}", [128, N, NSEG], f16)) for h in range(2)]
        NQB = [e(nc.sbuf_tensor(f"nq{b}", [128, QSEG, N], f32)) for b in range(2)]
        CB = [e(nc.sbuf_tensor(f"cb{b}", [128, LJW * NSEG], bf16))
              for b in range(NCB)]
        A = [e(nc.sbuf_tensor(f"a{h}", [128, NSEG], bf16)) for h in range(2)]
        K16 = e(nc.sbuf_tensor("k16", [128, N + 256], bf16))
        R = [e(nc.psum_tensor(f"r{h}", [128, NSEG], f32)) for h in range(2)]
        CP = [e(nc.psum_tensor(f"cp{b}", [128, LJW * NSEG], f32))
              for b in range(NCP)]

        e10sg = K16[:, 0:N]                # [128, 64]
        identb = K16[:, N : N + 128]       # [128, 128]
        bdb = K16[:, N + 128 : N + 256]    # [128, 128]

        sems = {
            "ink": e(nc.semaphore("sem_in_k")),
            "nq0": e(nc.semaphore("sem_nq0")),
            "nq1": e(nc.semaphore("sem_nq1")),
            "nv0": e(nc.semaphore("sem_nv0")),
            "nv1": e(nc.semaphore("sem_nv1")),
            "out": e(nc.semaphore("sem_out")),
            "outv": e(nc.semaphore("sem_outv")),
            "vector": e(nc.semaphore("sem_dve")),
            "scalar": e(nc.semaphore("sem_act")),
            "tensor": e(nc.semaphore("sem_pe")),
            "pool": e(nc.semaphore("sem_pool")),
        }

        P = Plan()

        # tick tables
        tE = {}      # ACT exp per quarter
        tP2 = {}     # gamma multiply per quarter -> (engine, tick)
        tI0 = {}     # pre invert exp per quarter
        tRSC0 = {}   # pre rowscale per quarter (DVE)
        tDV0 = {}    # pre divide per (Q, cc) (DVE)
        tINV = {}    # loop invert exp per (t, h) (ACT)
        tRSC = {}    # loop rowscale per (t, h) (DVE)
        tCV = {}     # loop convert per (t, h, c) (ACT)
        tDV = {}     # loop divide per (t, h, c) -> (engine, tick)
        cp_reader = [None] * NCP   # (engine, tick) that last read CP[b]
        cb_reader = [None] * NCB   # (engine, tick) that last read CB[b]

        XB = [X[h][:, :, :] for h in range(2)]     # base APs
        OB = [OUTB[h][:, :, :] for h in range(2)]

        # ---------------- global schedule walk -------------------------
        # sync: consts first
        P.add("sync", lambda: nc.sync.dma_start(out=K16[:, :], in_=k16_d[:, :])
              .then_inc(sems["ink"], 16))

        def plan_dma_in(Q):
            h, q = Q // 4, Q % 4
            w = [("scalar", tE[Q - 2])] if Q >= 2 else []
            for hh in range(2):
                def dma_in(h=h, hh=hh, q=q, b=Q % 2):
                    return nc.sync.dma_start(
                        out=NQB[b][hh * 64 : (hh + 1) * 64, :, :],
                        in_=noise_ap(h, hh, q),
                    ).then_inc(sems["nq%d" % (Q % 2)], 16)
                P.add("sync", dma_in, w)
                w = []

        # build ACT/DVE/PE/POOL streams quarter by quarter; ACT exp runs two
        # quarters ahead (ping-pong NQB).
        def plan_exp(Q):
            h, q = Q // 4, Q % 4
            # in: NQB[Q%2] [128, (seg 64), (j 64)]; out: X[h] quarter strided
            def emit(h=h, q=q, b=Q % 2):
                return nc.scalar.activation(
                    out=_mk(XB[h], q * QSEG, [[1, QSEG], [NSEG, N]]),
                    in_=NQB[b][:, :, :],
                    func=AF.Exp,
                    scale=1.0 / TEMP,
                    bias=-SHIFT,
                )
            tE[Q] = P.add("scalar", emit,
                          [("nq%d" % (Q % 2), 32 * (Q // 2 + 1))])

        def plan_quarter(Q):
            h, q = Q // 4, Q % 4
            qo = q * QSEG

            # P2: X_q *= e10sg  (j-major iteration; e10sg bcast over seg);
            # alternate GPSIMD/DVE so neither engine paces the pre-phase
            p2_pool = USE_POOL_P2 and (Q % 2 == 0)
            p2_eng = "pool" if p2_pool else "vector"
            def emit_p2(h=h, qo=qo, p2_pool=p2_pool):
                eng = nc.gpsimd if p2_pool else nc.vector
                xq = _mk(XB[h], qo, [[NSEG, N], [1, QSEG]])
                return eng.tensor_tensor(
                    out=xq, in0=xq,
                    in1=_mk(K16[:, 0:N], 0, [[1, N], [0, QSEG]]),
                    op=MUL,
                )
            tP2[Q] = (p2_eng, P.add(p2_eng, emit_p2,
                                    [("scalar", tE[Q]), ("ink", 16)]))

            # RS0: 64 accumulating matmuls -> R[h][:, qo:qo+QSEG]
            first = [("ink", 16), (tP2[Q][0], tP2[Q][1])]
            for j in range(N):
                def emit_rs0(h=h, qo=qo, j=j):
                    return nc.tensor.matmul(
                        R[h][:, qo : qo + QSEG],
                        identb,
                        _mk(XB[h], j * NSEG + qo, [[1, QSEG]]),
                        start=(j == 0),
                        stop=(j == N - 1),
                    )
                t_rs0 = P.add("tensor", emit_rs0, first)
                first = []

            # inv0: A_q = 1/R_q via DVE reciprocal (R0 up to ~2e31 exceeds the
            # ACT Ln domain of 2^64^0.5-ish; loop rowsums are tame)
            def emit_inv0(h=h, qo=qo):
                with nc.allow_low_precision("A0 feeds a bf16 multiply anyway"):
                    return nc.vector.reciprocal(
                        out=A[h][:, qo : qo + QSEG],
                        in_=R[h][:, qo : qo + QSEG],
                    )
            tI0[Q] = P.add("vector", emit_inv0, [("tensor", t_rs0)])

            # rsc0: X_q *= A_q (2x: j-major, seg-inner; A bcast over j)
            def emit_rsc0(h=h, qo=qo):
                xq = _mk(XB[h], qo, [[NSEG, N], [1, QSEG]])
                return nc.vector.tensor_tensor(
                    out=xq, in0=xq,
                    in1=_mk(A[h][:, :], qo, [[0, N], [1, QSEG]]),
                    op=MUL,
                )
            tRSC0[Q] = P.add("vector", emit_rsc0)

            # CS0 / cvt0 / div0 chunks: PJW=16 j-cols x QSEG=64 segs
            for cc in range(PCH):
                pb = cc % NCP
                cbb = cc % NCB
                w = [("vector", tRSC0[Q])] if cc == 0 else []
                if cp_reader[pb] is not None:
                    w.append(cp_reader[pb])
                for m in range(2):
                    def emit_cs0(h=h, qo=qo, cc=cc, m=m, pb=pb):
                        return nc.tensor.matmul(
                            _mk(CP[pb][:, :], m * 512, [[QSEG, 8], [1, QSEG]]),
                            bdb,
                            _mk(XB[h], (cc * PJW + 8 * m) * NSEG + qo,
                                [[NSEG, 8], [1, QSEG]]),
                            start=True, stop=True,
                        )
                    t_cs0 = P.add("tensor", emit_cs0, w)
                    w = []
                wcv = [("tensor", t_cs0)]
                if cb_reader[cbb] is not None:
                    wcv.append(cb_reader[cbb])
                if cc % 4 == 3:
                    def emit_ln0(pb=pb):
                        return nc.scalar.activation(
                            out=CP[pb][:, :], in_=CP[pb][:, :], func=AF.Ln,
                        )
                    P.add("scalar", emit_ln0, [("tensor", t_cs0)])
                    def emit_cv0(pb=pb, cbb=cbb):
                        return nc.scalar.activation(
                            out=CB[cbb][:, :], in_=CP[pb][:, :],
                            func=AF.Exp, scale=-1.0,
                        )
                    t_cv0 = P.add("scalar", emit_cv0, wcv)
                    inv_eng0 = "scalar"
                else:
                    def emit_cv0(pb=pb, cbb=cbb):
                        with nc.allow_low_precision("colsum recip to bf16"):
                            return nc.vector.reciprocal(
                                out=CB[cbb][:, :], in_=CP[pb][:, :],
                            )
                    t_cv0 = P.add("vector", emit_cv0, wcv)
                    inv_eng0 = "vector"
                cp_reader[pb] = (inv_eng0, t_cv0)
                def emit_dv0(h=h, qo=qo, cc=cc, cbb=cbb):
                    xq = _mk(XB[h], cc * PJW * NSEG + qo, [[NSEG, PJW], [1, QSEG]])
                    return nc.vector.tensor_tensor(
                        out=xq, in0=xq,
                        in1=_mk(CB[cbb][:, :], 0, [[QSEG, PJW], [1, QSEG]]),
                        op=MUL,
                    )
                tDV0[(Q, cc)] = P.add("vector", emit_dv0, [(inv_eng0, t_cv0)])
                cb_reader[cbb] = ("vector", tDV0[(Q, cc)])

        plan_dma_in(0)
        plan_exp(0)
        plan_dma_in(1)
        plan_exp(1)
        for Q in range(8):
            if Q + 2 < 8:
                plan_dma_in(Q + 2)
                plan_exp(Q + 2)
            plan_quarter(Q)

        # ---------------- main loop t = 0..18 --------------------------
        # Software-pipelined: rowsums for iteration t+1 are accumulated as
        # 4-matmul partials interleaved right behind iteration t's divides,
        # so the turnaround (last divide -> invert -> first rowscale) is
        # short and the two halves phase-shift against each other.
        act_inv_set = ACT_INV_SET
        pool_mul_set = POOL_MUL_SET
        tRSP = {}   # (t, h) -> PE tick of the last partial-rowsum matmul

        def plan_rsp(t, h, c, wdep):
            """Partial rowsums for iteration t: 4 matmuls for j in chunk c."""
            w = list(wdep)
            for jj in range(LJW):
                j = c * LJW + jj
                def emit_rsp(h=h, j=j, c=c, jj=jj):
                    return nc.tensor.matmul(
                        R[h][:, :], identb,
                        _mk(XB[h], j * NSEG, [[1, NSEG]]),
                        start=(c == 0 and jj == 0),
                        stop=(c == LCH - 1 and jj == LJW - 1),
                    )
                tick = P.add("tensor", emit_rsp, w, label=f"rsp{t},{h},{c}")
                w = []
            if c == LCH - 1:
                tRSP[(t, h)] = tick

        # rowsums for t=0 accumulate over the pre-phase divides
        for h in range(2):
            for c in range(LCH):
                plan_rsp(0, h, c, [("vector", tDV0[(h * 4 + 3, c // 4)])])

        # rowscale chunk widths in j-columns: small first chunk so the first
        # colsum (and with it the ACT convert stream) starts early
        RSCW = [4, 12, 16, 16, 16]
        RSCO = [0, 4, 16, 32, 48]          # cumulative j offsets
        def rsck_of(c):
            j = c * LJW
            for k in range(len(RSCW)):
                if j < RSCO[k] + RSCW[k]:
                    return k
            raise AssertionError

        for t in range(ITERS - 1):
            last = t == ITERS - 2
            for h in range(2):
                # invert: ln R in place, exp(-lnR) -> A (tame range in-loop)
                def emit_ln(h=h):
                    return nc.scalar.activation(
                        out=R[h][:, :], in_=R[h][:, :], func=AF.Ln,
                    )
                P.add("scalar", emit_ln, [("tensor", tRSP[(t, h)])], label=f"ln{t},{h}")
                def emit_inv(h=h):
                    return nc.scalar.activation(
                        out=A[h][:, :], in_=R[h][:, :], func=AF.Exp, scale=-1.0,
                    )
                tINV[(t, h)] = P.add("scalar", emit_inv, label=f"inv{t},{h}")

                w = [("scalar", tINV[(t, h)])]
                for k in range(len(RSCW)):
                    def emit_rsc(h=h, k=k):
                        xf = _mk(XB[h], RSCO[k] * NSEG, [[NSEG, RSCW[k]], [1, NSEG]])
                        return nc.vector.tensor_tensor(
                            out=xf, in0=xf,
                            in1=_mk(A[h][:, :], 0, [[0, RSCW[k]], [1, NSEG]]),
                            op=MUL,
                        )
                    tRSC[(t, h, k)] = P.add("vector", emit_rsc, w, label=f"rsc{t},{h},{k}")
                    w = []

                last_rsck = -1
                for c in range(LCH):
                    pb = c % NCP
                    cbb = c % NCB
                    w = []
                    if rsck_of(c) > last_rsck:
                        last_rsck = rsck_of(c)
                        w.append(("vector", tRSC[(t, h, last_rsck)]))
                    if cp_reader[pb] is not None:
                        w.append(cp_reader[pb])
                    for m in range(2):
                        def emit_cs(h=h, c=c, m=m, pb=pb):
                            return nc.tensor.matmul(
                                _mk(CP[pb][:, :], m * 512, [[NSEG, 2], [1, NSEG]]),
                                bdb,
                                _mk(XB[h], (c * LJW + 2 * m) * NSEG,
                                    [[NSEG, 2], [1, NSEG]]),
                                start=True, stop=True,
                            )
                        t_cs = P.add("tensor", emit_cs, w, label=f"cs{t},{h},{c}.{m}")
                        w = []

                    # destination: in-place X for t<18, OUT16 for t=18
                    if last:
                        dst = _mk(OB[h], c * LJW * NSEG, [[NSEG, LJW], [1, NSEG]])
                    else:
                        dst = _mk(XB[h], c * LJW * NSEG, [[NSEG, LJW], [1, NSEG]])
                    xin = _mk(XB[h], c * LJW * NSEG, [[NSEG, LJW], [1, NSEG]])
                    lbl = ("fdv" if last else "div") + f"{t},{h},{c}"

                    # invert the chunk's colsums into CB[cbb] (bf16)
                    wcv = [("tensor", t_cs)]
                    if cb_reader[cbb] is not None:
                        wcv.append(cb_reader[cbb])
                    if c in act_inv_set:
                        def emit_lnc(pb=pb):
                            return nc.scalar.activation(
                                out=CP[pb][:, :], in_=CP[pb][:, :], func=AF.Ln,
                            )
                        P.add("scalar", emit_lnc, [("tensor", t_cs)],
                              label=f"lnc{t},{h},{c}")
                        def emit_cv(pb=pb, cbb=cbb):
                            return nc.scalar.activation(
                                out=CB[cbb][:, :], in_=CP[pb][:, :],
                                func=AF.Exp, scale=-1.0,
                            )
                        t_cv = P.add("scalar", emit_cv, wcv, label=f"cvt{t},{h},{c}")
                        inv_eng = "scalar"
                    else:
                        def emit_cv(pb=pb, cbb=cbb):
                            with nc.allow_low_precision("colsum recip to bf16"):
                                return nc.vector.reciprocal(
                                    out=CB[cbb][:, :], in_=CP[pb][:, :],
                                )
                        t_cv = P.add("vector", emit_cv, wcv, label=f"cvt{t},{h},{c}")
                        inv_eng = "vector"
                    tCV[(t, h, c)] = (inv_eng, t_cv)
                    cp_reader[pb] = (inv_eng, t_cv)

                    mul_eng = "pool" if c in pool_mul_set else "vector"
                    def emit_dv(dst=dst, xin=xin, cbb=cbb, mul_eng=mul_eng):
                        eng = nc.gpsimd if mul_eng == "pool" else nc.vector
                        return eng.tensor_tensor(
                            out=dst, in0=xin,
                            in1=_mk(CB[cbb][:, :], 0, [[NSEG, LJW], [1, NSEG]]),
                            op=MUL,
                        )
                    tdv = P.add(mul_eng, emit_dv, [tCV[(t, h, c)]], label=lbl)
                    tDV[(t, h, c)] = (mul_eng, tdv)
                    cb_reader[cbb] = (mul_eng, tdv)

                    # interleave next iteration's partial rowsums far enough
                    # behind the divides that the CS->cvt->div pipeline depth
                    # (~6 chunks) never stalls the PE stream
                    if not last and c >= RSP_LAG:
                        plan_rsp(t + 1, h, c - RSP_LAG, [tDV[(t, h, c - RSP_LAG)]])
                if not last:
                    for cc2 in range(LCH - RSP_LAG, LCH):
                        plan_rsp(t + 1, h, cc2, [tDV[(t, h, cc2)]])

        # output DMA: one raw transfer per half
        for h in range(2):
            # wait the last divide on each engine (DVE odd, pool even chunks)
            w = [tDV[(ITERS - 2, h, LCH - 1)], tDV[(ITERS - 2, h, LCH - 2)]]
            def dma_out(h=h):
                return nc.sync.dma_start(
                    out=out_ap(h),
                    in_=OUTB[h][:, :, :],
                ).then_inc(sems["out"], 16)
            P.add("sync", dma_out, w)
        P.add("sync", lambda: None, [("out", 32)])

        # ------------------------------------------------------------------
        # emission
        # ------------------------------------------------------------------
        def emit_stream(handle, eng):
            own = sems.get(eng)
            waited = {}
            tick = 0
            self_order = eng in ("vector", "scalar", "pool")
            for emit, waits, label, counted in P.ops[eng]:
                for sname, val in waits:
                    if waited.get(sname, 0) >= val:
                        continue
                    handle.wait_ge(sems[sname], val)
                    waited[sname] = val
                if self_order and tick > 0 and waited.get(eng, 0) < tick:
                    handle.wait_ge(sems[eng], tick)
                    waited[eng] = tick
                inst = emit()
                if not counted:
                    continue
                tick += 1
                if eng != "sync" and inst is not None:
                    inst.then_inc(sems[eng], 1)
                    if label:
                        DEBUG_LABELS[inst.ins.name] = label

        with nc.Block() as block:

            @block.sync
            def _(sync):
                emit_stream(sync, "sync")

            @block.vector
            def _(vector):
                emit_stream(vector, "vector")

            @block.scalar
            def _(scalar):
                emit_stream(scalar, "scalar")

            @block.tensor
            def _(tensor):
                emit_stream(tensor, "tensor")

            if P.ops["pool"]:

                @block.gpsimd
                def _(pool):
                    emit_stream(pool, "pool")

    return nc


def host_constants():
    """[128, 64+256] bf16: e10sg placeholder | identity | block-diag."""
    import ml_dtypes

    ident = np.eye(128, dtype=np.float32)
    bdiag = np.kron(np.eye(2, dtype=np.float32), np.ones((64, 64), np.float32))
    return ident, bdiag


def pack_consts16(gamma):
    import ml_dtypes

    sg = 1.0 / (1.0 + np.exp(-gamma.astype(np.float64)))
    e10sg = np.exp(sg / TEMP).astype(np.float32)           # [64, 64]
    e10sg2 = np.concatenate([e10sg, e10sg], axis=0)        # [128, 64]
    ident, bdiag = host_constants()
    k = np.concatenate([e10sg2, ident, bdiag], axis=1)     # [128, 320]
    return k.astype(ml_dtypes.bfloat16)


def kernel(gamma: np.ndarray, gumbel_noise: np.ndarray) -> np.ndarray:
    from concourse.bass_utils import run_bass_kernel_spmd

    gamma = np.asarray(gamma, dtype=np.float32)
    noise = np.asarray(gumbel_noise, dtype=np.float32)
    s = noise.shape[0]
    s_per_core = s // NCORES
    if s_per_core not in _PROGRAM_CACHE:
        _PROGRAM_CACHE[s_per_core] = build_program(s_per_core=s_per_core)
    nc = _PROGRAM_CACHE[s_per_core]

    k16 = pack_consts16(gamma)
    in_maps = []
    for c in range(NCORES):
        shard = np.ascontiguousarray(noise[c * s_per_core : (c + 1) * s_per_core])
        in_maps.append({"noise": shard, "consts16": k16})
    res = run_bass_kernel_spmd(nc, in_maps, list(range(NCORES)))
    outs = []
    for r in res.results:
        buf = np.asarray(r["out"])  # [2, 128, 64, 256] = (h, (hh,i), j, seg)
        buf = buf.reshape(2, 2, 64, N, NSEG).transpose(0, 1, 4, 2, 3)
        outs.append(buf.reshape(s_per_core, N, N))
    return np.concatenate(outs, axis=0).astype(np.float32)


# revision 53
# speedup vs baseline: 1.5331x; 1.0105x over previous
"""Gumbel-Sinkhorn kernel for Trainium2 (raw Bass, manual sems) — v2.

Math: per sample, L = (sigmoid(gamma) + noise)/temp, then 20 iterations of
row-logsumexp-subtract / col-logsumexp-subtract, output exp(result).  In
linear space that is Sinkhorn scaling of X0 = exp(L - S) (S = 80 constant
shift; safe: data exponent range [-24.4, 144.4], per-row max >= 20.9, so
sums stay below fp32 max and no significant entry underflows bf16).

v2 design (vs v1 baseline, TimelineSim ~1.4 ms vs ~4.3 ms):
  * X stored bf16 with free layout (j-outer, seg-inner).  Both elementwise
    passes per iteration then qualify for the DVE 2x_1p mode (2-byte dtype,
    stride-1 innermost on every operand; the broadcast operand puts its
    stride-0 dim outermost).  fp16 is impossible: entries that end up O(1)
    dip to ~1e-17 mid-iteration, below fp16 range.
  * All loop matmuls are bf16 (1 cycle/row vs fp32's 4 on the PE).
  * Col normalization is invert-then-multiply, chunked 4 j-columns at a
    time and load-balanced across engines: each chunk's colsum reciprocal
    comes from DVE InstReciprocal (PSUM -> bf16, chunks not in ACT_INV_SET)
    or ACT ln + exp(-x) (ACT_INV_SET), and the multiply runs at DVE 2x or
    on GPSIMD (POOL_MUL_SET).  (TensorTensor divide is not a valid DVE ISA
    op, and GPSIMD cannot access PSUM — see memory notes.)
  * Rowsums for iteration t+1 are accumulated as 4-matmul partials
    interleaved RSP_LAG chunks behind iteration t's multiplies, so the
    turnaround (last multiply -> invert -> first rowscale chunk) is short;
    the two halves phase-shift against each other.
  * The exp is a single ACT pass per quarter with scale=10, bias=-80;
    exp(10*sigmoid(gamma)) is folded in as a separate multiply on
    GPSIMD/DVE (alternating per quarter).
  * Output is written fp16 in the on-chip layout and DMA'd as one raw
    32 KiB-run transfer per half (runs < 512 B pay a 2x DMA latency
    penalty); the host reorders and casts.

Iteration structure: pre-phase does exp, rowsums0, rowscale0, colsums0 and
col-norm0 per 256-sample quarter (pipelined with the input DMA); the main
loop runs t = 0..18 of [partial rowsums, invert, rowscale(chunked),
colsums(chunked), colsum-invert, multiply], with t = 18 writing the
multiplies to the output buffer.

Sharding: pure data parallel over samples -> 1024 per core, SPMD on 8 cores.
"""

import sys

if "/opt/trn_rl_repo" not in sys.path:
    sys.path.insert(0, "/opt/trn_rl_repo")

import numpy as np

N = 64
ITERS = 20
TEMP = 0.1
SHIFT = 80.0
NUM_SAMPLES = 8192
NCORES = 8
S_PER_CORE = NUM_SAMPLES // NCORES  # 1024
NSEG = 256          # samples per (half, hh-block)
NQ = 4              # quarters per half
QSEG = NSEG // NQ   # 64 segs per quarter

# chunking of the col path in the main loop: 16 chunks of 4 j-columns
LCH = 16
LJW = N // LCH      # 4
# chunking in the pre-phase quarters: 4 chunks of 16 j-columns x 64 segs
PCH = 4
PJW = N // PCH      # 16

USE_POOL_P2 = True       # exp(10*sigmoid(gamma)) multiply on GPSIMD
# middle chunks go to GPSIMD: early chunks gate the PE rowsum-partial stream,
# the tail chunk gates the next invert, so both stay on the faster engines
# Hardware rules discovered the hard way: GPSIMD cannot touch PSUM, and
# TensorTensor divide is not a valid DVE ISA op.  The col normalization is
# therefore invert-then-multiply: each chunk's colsum reciprocal comes from
# either DVE InstReciprocal (PSUM -> bf16 CB directly) or ACT ln/exp
# (in-place PSUM ln, then exp(-lnC) -> CB), and the multiply runs at DVE 2x
# or on GPSIMD from SBUF.
ACT_INV_SET = {0, 1, 3, 5, 9, 13, 15}         # chunks inverted via ACT ln/exp
POOL_MUL_SET = {1, 2, 4, 5, 6, 8, 9, 10, 12, 13}   # multiply chunks on GPSIMD
RSP_LAG = 9                       # chunks the rowsum partials trail the divides

_PROGRAM_CACHE = {}
DEBUG_LABELS = {}


def _mk(base, extra_off, free_dims):
    """AP with base's partition dim, custom free dims ([stride, count] elems)."""
    import concourse.bass as bass

    return bass.AP(
        tensor=base.tensor,
        offset=base.offset + extra_off,
        ap=[list(base.ap[0])] + [list(d) for d in free_dims],
    )


def build_program(s_per_core=S_PER_CORE):
    from contextlib import ExitStack

    import concourse.bass as bass
    from concourse import mybir

    f32 = mybir.dt.float32
    bf16 = mybir.dt.bfloat16
    f16 = mybir.dt.float16
    AF = mybir.ActivationFunctionType
    DIV = mybir.AluOpType.divide
    MUL = mybir.AluOpType.mult

    assert s_per_core == 1024, "layout hardcoded for 1024 samples/core"

    nc = bass.Bass()
    # register a [128,1] const AP for the exp bias (only 0.0/1.0 are built in)
    _bias_t = nc.alloc_sbuf_tensor(f"const-f32-bias", [128, 1], f32)
    nc.gpsimd.memset(_bias_t.ap(), -SHIFT)
    nc.const_aps.aps[(f32, -SHIFT)] = _bias_t.ap()
    nc.all_engine_barrier()

    noise_d = nc.dram_tensor("noise", [s_per_core, N, N], f32, kind="ExternalInput")
    k16_d = nc.dram_tensor("consts16", [128, N + 256], bf16, kind="ExternalInput")
    # raw dump in the on-chip (h, p, j, seg) layout; host reorders.  32 KiB
    # contiguous runs keep the output DMA at full bandwidth (runs < 512 B
    # pay a 2x latency penalty in the DMA engines).
    out_d = nc.dram_tensor("out", [2, 128, N, NSEG], f16, kind="ExternalOutput")

    def noise_ap(h, hh, q):
        base = (h * 512 + hh * 256 + q * QSEG) * N * N
        return bass.AP(
            tensor=noise_d.tensor if hasattr(noise_d, "tensor") else noise_d,
            offset=base,
            ap=[[N, N], [N * N, QSEG], [1, N]],
        )

    def out_ap(h):
        base = h * 128 * N * NSEG
        return bass.AP(
            tensor=out_d.tensor if hasattr(out_d, "tensor") else out_d,
            offset=base,
            ap=[[N * NSEG, 128], [1, N * NSEG]],
        )

    # ------------------------------------------------------------------
    # planning: per-engine op lists with cross-engine tick waits
    # ------------------------------------------------------------------
    ENGINES = ("sync", "vector", "scalar", "tensor", "pool")

    class Plan:
        def __init__(self):
            self.ops = {e: [] for e in ENGINES}
            self.n = {e: 0 for e in ENGINES}

        def add(self, eng, emit, waits=(), label="", counted=True):
            self.ops[eng].append((emit, [w for w in waits if w is not None],
                                  label, counted))
            if counted:
                self.n[eng] += 1
            return self.n[eng]

    with ExitStack() as ctx:
        e = ctx.enter_context
        NCB = 8
        NCP = 3
        X = [e(nc.sbuf_tensor(f"x{h}", [128, N, NSEG], bf16)) for h in range(2)]
        OUTB = [e(nc.sbuf_tensor(f"ob{h}", [128, N, NSEG], f16)) for h in range(2)]
        NQB = [e(nc.sbuf_tensor(f"nq{b}", [128, QSEG, N], f32)) for b in range(2)]
        CB = [e(nc.sbuf_tensor(f"cb{b}", [128, LJW * NSEG], bf16))
              for b in range(NCB)]
        A = [e(nc.sbuf_tensor(f"a{h}", [128, NSEG], bf16)) for h in range(2)]
        K16 = e(nc.sbuf_tensor("k16", [128, N + 256], bf16))
        R = [e(nc.psum_tensor(f"r{h}", [128, NSEG], f32)) for h in range(2)]
        CP = [e(nc.psum_tensor(f"cp{b}", [128, LJW * NSEG], f32))
              for b in range(NCP)]

        e10sg = K16[:, 0:N]                # [128, 64]
        identb = K16[:, N : N + 128]       # [128, 128]
        bdb = K16[:, N + 128 : N + 256]    # [128, 128]

        sems = {
            "ink": e(nc.semaphore("sem_in_k")),
            "nq0": e(nc.semaphore("sem_nq0")),
            "nq1": e(nc.semaphore("sem_nq1")),
            "nv0": e(nc.semaphore("sem_nv0")),
            "nv1": e(nc.semaphore("sem_nv1")),
            "out": e(nc.semaphore("sem_out")),
            "outv": e(nc.semaphore("sem_outv")),
            "vector": e(nc.semaphore("sem_dve")),
            "scalar": e(nc.semaphore("sem_act")),
            "tensor": e(nc.semaphore("sem_pe")),
            "pool": e(nc.semaphore("sem_pool")),
        }

        P = Plan()

        # tick tables
        tE = {}      # ACT exp per quarter
        tP2 = {}     # gamma multiply per quarter -> (engine, tick)
        tI0 = {}     # pre invert exp per quarter
        tRSC0 = {}   # pre rowscale per quarter (DVE)
        tDV0 = {}    # pre divide per (Q, cc) (DVE)
        tINV = {}    # loop invert exp per (t, h) (ACT)
        tRSC = {}    # loop rowscale per (t, h) (DVE)
        tCV = {}     # loop convert per (t, h, c) (ACT)
        tDV = {}     # loop divide per (t, h, c) -> (engine, tick)
        cp_reader = [None] * NCP   # (engine, tick) that last read CP[b]
        cb_reader = [None] * NCB   # (engine, tick) that last read CB[b]

        XB = [X[h][:, :, :] for h in range(2)]     # base APs
        OB = [OUTB[h][:, :, :] for h in range(2)]

        # ---------------- global schedule walk -------------------------
        # sync: consts first
        P.add("sync", lambda: nc.sync.dma_start(out=K16[:, :], in_=k16_d[:, :])
              .then_inc(sems["ink"], 16))

        def plan_dma_in(Q):
            h, q = Q // 4, Q % 4
            w = [("scalar", tE[Q - 2])] if Q >= 2 else []
            for hh in range(2):
                def dma_in(h=h, hh=hh, q=q, b=Q % 2):
                    return nc.sync.dma_start(
                        out=NQB[b][hh * 64 : (hh + 1) * 64, :, :],
                        in_=noise_ap(h, hh, q),
                    ).then_inc(sems["nq%d" % (Q % 2)], 16)
                P.add("sync", dma_in, w)
                w = []

        # build ACT/DVE/PE/POOL streams quarter by quarter; ACT exp runs two
        # quarters ahead (ping-pong NQB).
        def plan_exp(Q):
            h, q = Q // 4, Q % 4
            # in: NQB[Q%2] [128, (seg 64), (j 64)]; out: X[h] quarter strided
            def emit(h=h, q=q, b=Q % 2):
                return nc.scalar.activation(
                    out=_mk(XB[h], q * QSEG, [[1, QSEG], [NSEG, N]]),
                    in_=NQB[b][:, :, :],
                    func=AF.Exp,
                    scale=1.0 / TEMP,
                    bias=-SHIFT,
                )
            tE[Q] = P.add("scalar", emit,
                          [("nq%d" % (Q % 2), 32 * (Q // 2 + 1))])

        def plan_quarter(Q):
            h, q = Q // 4, Q % 4
            qo = q * QSEG

            # P2: X_q *= e10sg  (j-major iteration; e10sg bcast over seg);
            # alternate GPSIMD/DVE so neither engine paces the pre-phase
            p2_pool = USE_POOL_P2 and (Q % 2 == 0)
            p2_eng = "pool" if p2_pool else "vector"
            def emit_p2(h=h, qo=qo, p2_pool=p2_pool):
                eng = nc.gpsimd if p2_pool else nc.vector
                xq = _mk(XB[h], qo, [[NSEG, N], [1, QSEG]])
                return eng.tensor_tensor(
                    out=xq, in0=xq,
                    in1=_mk(K16[:, 0:N], 0, [[1, N], [0, QSEG]]),
                    op=MUL,
                )
            tP2[Q] = (p2_eng, P.add(p2_eng, emit_p2,
                                    [("scalar", tE[Q]), ("ink", 16)]))

            # RS0: 64 accumulating matmuls -> R[h][:, qo:qo+QSEG]
            first = [("ink", 16), (tP2[Q][0], tP2[Q][1])]
            for j in range(N):
                def emit_rs0(h=h, qo=qo, j=j):
                    return nc.tensor.matmul(
                        R[h][:, qo : qo + QSEG],
                        identb,
                        _mk(XB[h], j * NSEG + qo, [[1, QSEG]]),
                        start=(j == 0),
                        stop=(j == N - 1),
                    )
                t_rs0 = P.add("tensor", emit_rs0, first)
                first = []

            # inv0: A_q = 1/R_q via DVE reciprocal (R0 up to ~2e31 exceeds the
            # ACT Ln domain of 2^64^0.5-ish; loop rowsums are tame)
            def emit_inv0(h=h, qo=qo):
                with nc.allow_low_precision("A0 feeds a bf16 multiply anyway"):
                    return nc.vector.reciprocal(
                        out=A[h][:, qo : qo + QSEG],
                        in_=R[h][:, qo : qo + QSEG],
                    )
            tI0[Q] = P.add("vector", emit_inv0, [("tensor", t_rs0)])

            # rsc0: X_q *= A_q (2x: j-major, seg-inner; A bcast over j)
            def emit_rsc0(h=h, qo=qo):
                xq = _mk(XB[h], qo, [[NSEG, N], [1, QSEG]])
                return nc.vector.tensor_tensor(
                    out=xq, in0=xq,
                    in1=_mk(A[h][:, :], qo, [[0, N], [1, QSEG]]),
                    op=MUL,
                )
            tRSC0[Q] = P.add("vector", emit_rsc0)

            # CS0 / cvt0 / div0 chunks: PJW=16 j-cols x QSEG=64 segs
            for cc in range(PCH):
                pb = cc % NCP
                cbb = cc % NCB
                w = [("vector", tRSC0[Q])] if cc == 0 else []
                if cp_reader[pb] is not None:
                    w.append(cp_reader[pb])
                for m in range(2):
                    def emit_cs0(h=h, qo=qo, cc=cc, m=m, pb=pb):
                        return nc.tensor.matmul(
                            _mk(CP[pb][:, :], m * 512, [[QSEG, 8], [1, QSEG]]),
                            bdb,
                            _mk(XB[h], (cc * PJW + 8 * m) * NSEG + qo,
                                [[NSEG, 8], [1, QSEG]]),
                            start=True, stop=True,
                        )
                    t_cs0 = P.add("tensor", emit_cs0, w)
                    w = []
                wcv = [("tensor", t_cs0)]
                if cb_reader[cbb] is not None:
                    wcv.append(cb_reader[cbb])
                if cc % 4 == 3:
                    def emit_ln0(pb=pb):
                        return nc.scalar.activation(
                            out=CP[pb][:, :], in_=CP[pb][:, :], func=AF.Ln,
                        )
                    P.add("scalar", emit_ln0, [("tensor", t_cs0)])
                    def emit_cv0(pb=pb, cbb=cbb):
                        return nc.scalar.activation(
                            out=CB[cbb][:, :], in_=CP[pb][:, :],
                            func=AF.Exp, scale=-1.0,
                        )
                    t_cv0 = P.add("scalar", emit_cv0, wcv)
                    inv_eng0 = "scalar"
                else:
                    def emit_cv0(pb=pb, cbb=cbb):
                        with nc.allow_low_precision("colsum recip to bf16"):
                            return nc.vector.reciprocal(
                                out=CB[cbb][:, :], in_=CP[pb][:, :],
                            )
                    t_cv0 = P.add("vector", emit_cv0, wcv)
                    inv_eng0 = "vector"
                cp_reader[pb] = (inv_eng0, t_cv0)
                def emit_dv0(h=h, qo=qo, cc=cc, cbb=cbb):
                    xq = _mk(XB[h], cc * PJW * NSEG + qo, [[NSEG, PJW], [1, QSEG]])
                    return nc.vector.tensor_tensor(
                        out=xq, in0=xq,
                        in1=_mk(CB[cbb][:, :], 0, [[QSEG, PJW], [1, QSEG]]),
                        op=MUL,
                    )
                tDV0[(Q, cc)] = P.add("vector", emit_dv0, [(inv_eng0, t_cv0)])
                cb_reader[cbb] = ("vector", tDV0[(Q, cc)])

        plan_dma_in(0)
        plan_exp(0)
        plan_dma_in(1)
        plan_exp(1)
        for Q in range(8):
            if Q + 2 < 8:
                plan_dma_in(Q + 2)
                plan_exp(Q + 2)
            plan_quarter(Q)

        # ---------------- main loop t = 0..18 --------------------------
        # Software-pipelined: rowsums for iteration t+1 are accumulated as
        # 4-matmul partials interleaved right behind iteration t's divides,
        # so the turnaround (last divide -> invert -> first rowscale) is
        # short and the two halves phase-shift against each other.
        act_inv_set = ACT_INV_SET
        pool_mul_set = POOL_MUL_SET
        tRSP = {}   # (t, h) -> PE tick of the last partial-rowsum matmul

        def plan_rsp(t, h, c, wdep):
            """Partial rowsums for iteration t: 4 matmuls for j in chunk c."""
            w = list(wdep)
            for jj in range(LJW):
                j = c * LJW + jj
                def emit_rsp(h=h, j=j, c=c, jj=jj):
                    return nc.tensor.matmul(
                        R[h][:, :], identb,
                        _mk(XB[h], j * NSEG, [[1, NSEG]]),
                        start=(c == 0 and jj == 0),
                        stop=(c == LCH - 1 and jj == LJW - 1),
                    )
                tick = P.add("tensor", emit_rsp, w, label=f"rsp{t},{h},{c}")
                w = []
            if c == LCH - 1:
                tRSP[(t, h)] = tick

        # rowsums for t=0 accumulate over the pre-phase divides
        for h in range(2):
            for c in range(LCH):
                plan_rsp(0, h, c, [("vector", tDV0[(h * 4 + 3, c // 4)])])

        # rowscale chunk widths in j-columns: small first chunk so the first
        # colsum (and with it the ACT convert stream) starts early
        RSCW = [4, 12, 16, 16, 16]
        RSCO = [0, 4, 16, 32, 48]          # cumulative j offsets
        def rsck_of(c):
            j = c * LJW
            for k in range(len(RSCW)):
                if j < RSCO[k] + RSCW[k]:
                    return k
            raise AssertionError

        for t in range(ITERS - 1):
            last = t == ITERS - 2
            for h in range(2):
                # invert: ln R in place, exp(-lnR) -> A (tame range in-loop)
                def emit_ln(h=h):
                    return nc.scalar.activation(
                        out=R[h][:, :], in_=R[h][:, :], func=AF.Ln,
                    )
                P.add("scalar", emit_ln, [("tensor", tRSP[(t, h)])], label=f"ln{t},{h}")
                def emit_inv(h=h):
                    return nc.scalar.activation(
                        out=A[h][:, :], in_=R[h][:, :], func=AF.Exp, scale=-1.0,
                    )
                tINV[(t, h)] = P.add("scalar", emit_inv, label=f"inv{t},{h}")

                w = [("scalar", tINV[(t, h)])]
                for k in range(len(RSCW)):
                    def emit_rsc(h=h, k=k):
                        xf = _mk(XB[h], RSCO[k] * NSEG, [[NSEG, RSCW[k]], [1, NSEG]])
                        return nc.vector.tensor_tensor(
                            out=xf, in0=xf,
                            in1=_mk(A[h][:, :], 0, [[0, RSCW[k]], [1, NSEG]]),
                            op=MUL,
                        )
                    tRSC[(t, h, k)] = P.add("vector", emit_rsc, w, label=f"rsc{t},{h},{k}")
                    w = []

                last_rsck = -1
                for c in range(LCH):
                    pb = c % NCP
                    cbb = c % NCB
                    w = []
                    if rsck_of(c) > last_rsck:
                        last_rsck = rsck_of(c)
                        w.append(("vector", tRSC[(t, h, last_rsck)]))
                    if cp_reader[pb] is not None:
                        w.append(cp_reader[pb])
                    for m in range(2):
                        def emit_cs(h=h, c=c, m=m, pb=pb):
                            return nc.tensor.matmul(
                                _mk(CP[pb][:, :], m * 512, [[NSEG, 2], [1, NSEG]]),
                                bdb,
                                _mk(XB[h], (c * LJW + 2 * m) * NSEG,
                                    [[NSEG, 2], [1, NSEG]]),
                                start=True, stop=True,
                            )
                        t_cs = P.add("tensor", emit_cs, w, label=f"cs{t},{h},{c}.{m}")
                        w = []

                    # destination: in-place X for t<18, OUT16 for t=18
                    if last:
                        dst = _mk(OB[h], c * LJW * NSEG, [[NSEG, LJW], [1, NSEG]])
                    else:
                        dst = _mk(XB[h], c * LJW * NSEG, [[NSEG, LJW], [1, NSEG]])
                    xin = _mk(XB[h], c * LJW * NSEG, [[NSEG, LJW], [1, NSEG]])
                    lbl = ("fdv" if last else "div") + f"{t},{h},{c}"

                    # invert the chunk's colsums into CB[cbb] (bf16)
                    wcv = [("tensor", t_cs)]
                    if cb_reader[cbb] is not None:
                        wcv.append(cb_reader[cbb])
                    if c in act_inv_set:
                        def emit_lnc(pb=pb):
                            return nc.scalar.activation(
                                out=CP[pb][:, :], in_=CP[pb][:, :], func=AF.Ln,
                            )
                        P.add("scalar", emit_lnc, [("tensor", t_cs)],
                              label=f"lnc{t},{h},{c}")
                        def emit_cv(pb=pb, cbb=cbb):
                            return nc.scalar.activation(
                                out=CB[cbb][:, :], in_=CP[pb][:, :],
                                func=AF.Exp, scale=-1.0,
                            )
                        t_cv = P.add("scalar", emit_cv, wcv, label=f"cvt{t},{h},{c}")
                        inv_eng = "scalar"
                    else:
                        def emit_cv(pb=pb, cbb=cbb):
                            with nc.allow_low_precision("colsum recip to bf16"):
                                return nc.vector.reciprocal(
                                    out=CB[cbb][:, :], in_=CP[pb][:, :],
                                )
                        t_cv = P.add("vector", emit_cv, wcv, label=f"cvt{t},{h},{c}")
                        inv_eng = "vector"
                    tCV[(t, h, c)] = (inv_eng, t_cv)
                    cp_reader[pb] = (inv_eng, t_cv)

                    mul_eng = "pool" if c in pool_mul_set else "vector"
                    def emit_dv(dst=dst, xin=xin, cbb=cbb, mul_eng=mul_eng):
                        eng = nc.gpsimd if mul_eng == "pool" else nc.vector
                        return eng.tensor_tensor(
                            out=dst, in0=xin,
                            in1=_mk(CB[cbb][:, :], 0, [[NSEG, LJW], [1, NSEG]]),
                            op=MUL,
                        )
                    tdv = P.add(mul_eng, emit_dv, [tCV[(t, h, c)]], label=lbl)
                    tDV[(t, h, c)] = (mul_eng, tdv)
                    cb_reader[cbb] = (mul_eng, tdv)

                    # interleave next iteration's partial rowsums far enough
                    # behind the divides that the CS->cvt->div pipeline depth
                    # (~6 chunks) never stalls the PE stream
                    if not last and c >= RSP_LAG:
                        plan_rsp(t + 1, h, c - RSP_LAG, [tDV[(t, h, c - RSP_LAG)]])
                if not last:
                    for cc2 in range(LCH - RSP_LAG, LCH):
                        plan_rsp(t + 1, h, cc2, [tDV[(t, h, cc2)]])

        # output DMA: one raw transfer per half
        for h in range(2):
            # wait the last OUT16-writing multiply on each engine
            last_tick = {}
            for c in range(LCH):
                eng, tick = tDV[(ITERS - 2, h, c)]
                last_tick[eng] = max(last_tick.get(eng, 0), tick)
            w = list(last_tick.items())
            def dma_out(h=h):
                return nc.sync.dma_start(
                    out=out_ap(h),
                    in_=OUTB[h][:, :, :],
                ).then_inc(sems["out"], 16)
            P.add("sync", dma_out, w)
        P.add("sync", lambda: None, [("out", 32)])

        # ------------------------------------------------------------------
        # emission
        # ------------------------------------------------------------------
        def emit_stream(handle, eng):
            own = sems.get(eng)
            waited = {}
            tick = 0
            self_order = eng in ("vector", "scalar", "pool")
            for emit, waits, label, counted in P.ops[eng]:
                for sname, val in waits:
                    if waited.get(sname, 0) >= val:
                        continue
                    handle.wait_ge(sems[sname], val)
                    waited[sname] = val
                if self_order and tick > 0 and waited.get(eng, 0) < tick:
                    handle.wait_ge(sems[eng], tick)
                    waited[eng] = tick
                inst = emit()
                if not counted:
                    continue
                tick += 1
                if eng != "sync" and inst is not None:
                    inst.then_inc(sems[eng], 1)
                    if label:
                        DEBUG_LABELS[inst.ins.name] = label

        with nc.Block() as block:

            @block.sync
            def _(sync):
                emit_stream(sync, "sync")

            @block.vector
            def _(vector):
                emit_stream(vector, "vector")

            @block.scalar
            def _(scalar):
                emit_stream(scalar, "scalar")

            @block.tensor
            def _(tensor):
                emit_stream(tensor, "tensor")

            if P.ops["pool"]:

                @block.gpsimd
                def _(pool):
                    emit_stream(pool, "pool")

    return nc


def host_constants():
    """[128, 64+256] bf16: e10sg placeholder | identity | block-diag."""
    import ml_dtypes

    ident = np.eye(128, dtype=np.float32)
    bdiag = np.kron(np.eye(2, dtype=np.float32), np.ones((64, 64), np.float32))
    return ident, bdiag


def pack_consts16(gamma):
    import ml_dtypes

    sg = 1.0 / (1.0 + np.exp(-gamma.astype(np.float64)))
    e10sg = np.exp(sg / TEMP).astype(np.float32)           # [64, 64]
    e10sg2 = np.concatenate([e10sg, e10sg], axis=0)        # [128, 64]
    ident, bdiag = host_constants()
    k = np.concatenate([e10sg2, ident, bdiag], axis=1)     # [128, 320]
    return k.astype(ml_dtypes.bfloat16)


def kernel(gamma: np.ndarray, gumbel_noise: np.ndarray) -> np.ndarray:
    from concourse.bass_utils import run_bass_kernel_spmd

    gamma = np.asarray(gamma, dtype=np.float32)
    noise = np.asarray(gumbel_noise, dtype=np.float32)
    s = noise.shape[0]
    s_per_core = s // NCORES
    if s_per_core not in _PROGRAM_CACHE:
        _PROGRAM_CACHE[s_per_core] = build_program(s_per_core=s_per_core)
    nc = _PROGRAM_CACHE[s_per_core]

    k16 = pack_consts16(gamma)
    in_maps = []
    for c in range(NCORES):
        shard = np.ascontiguousarray(noise[c * s_per_core : (c + 1) * s_per_core])
        in_maps.append({"noise": shard, "consts16": k16})
    res = run_bass_kernel_spmd(nc, in_maps, list(range(NCORES)))
    outs = []
    for r in res.results:
        buf = np.asarray(r["out"])  # [2, 128, 64, 256] = (h, (hh,i), j, seg)
        buf = buf.reshape(2, 2, 64, N, NSEG).transpose(0, 1, 4, 2, 3)
        outs.append(buf.reshape(s_per_core, N, N))
    return np.concatenate(outs, axis=0).astype(np.float32)
